# revision 10
# baseline (speedup 1.0000x reference)
"""Neural Factorization Machine — Trainium2 Bass kernel, 8 NeuronCores.

Math (see reference):
    sum_emb = x @ emb; sum_sq = (x*x) @ (emb*emb)
    iv      = 0.5 * (sum_emb^2 - sum_sq)               [B, D]
    h       = relu(iv @ w1.T + b1)                     [B, D]
    inter   = h @ w2.T + b2                            [B, O]
    linear  = x @ lin_w.T + lin_b                      [B, O]
    out     = linear + inter
Returns (out, linear, inter) like the reference.

Sharding (8 cores, core c -> g = c//4 batch half, j = c%4 O-quarter):
  - big linear: rows g*512:(g+1)*512, cols j*1024:(j+1)*1024  (2x4 grid)
  - FM partial sums: core j of each half reduces F-slice j*4096:(j+1)*4096,
    AllReduce(add) over groups [[0..3],[4..7]] completes sum_emb/sum_sq
  - MLP replicated per half (tiny); second layer column-sharded by j.

Precision: the streamed operands (x, lin_w, emb) are bf16 (host cast,
fp32 PSUM accumulation) — halves HBM traffic at the same PE rate (fp32r
already ran 1 cycle/row).  Outputs are stored bf16 and upcast on the
host.  AllReduce payload and the small MLP stay fp32/f32r.

Layout: every streamed tensor is host-permuted so that one DMA moves one
contiguous >=4KB chunk per SBUF partition (smaller descriptors run at
half DMA bandwidth).  Engine queues are specialized so the weight stream
(sync) and x stream (gpsimd) are never blocked behind the collective or
the MLP prefetches (scalar queue); the AllReduce is emitted mid-C so its
latency hides behind the big matmul, and iv/h are precomputed on
vector/scalar so the PE rolls from C's last matmul straight into B.
"""

import sys

for _p in ("/opt/trn_rl_repo",):
    if _p not in sys.path:
        sys.path.append(_p)

import numpy as np
import ml_dtypes

BF16 = ml_dtypes.bfloat16

B, F, D, O = 1024, 16384, 256, 4096
NCORES = 8
GROUPS = [[0, 1, 2, 3], [4, 5, 6, 7]]
Bc, Oc, Fj = B // 2, O // 4, F // 4   # per-core: 512 batch rows, 1024 O cols, 4096 F slice
P = 128
KT_C = F // P      # 128 k-tiles, big linear
KT_A = Fj // P     # 32 k-tiles, FM partials
G_C = 4            # k-tiles per DMA group, phase C
G_A = 4            # k-tiles per DMA group, phase A
NG_C = KT_C // G_C
NG_A = KT_A // G_A

_cache = {}


def _group_rows(a, g):
    """[KT*P, C] -> [KT//g, P, g*C]: DMA group kg hands partition p one
    contiguous g*C-element line (k-tiles kg*g..kg*g+g-1, partition row p)."""
    kt = a.shape[0] // P
    c = a.shape[1]
    return np.ascontiguousarray(
        a.reshape(kt // g, g, P, c).transpose(0, 2, 1, 3).reshape(kt // g, P, g * c))


def _part_rows(a):
    """[KT*P, C] -> [P, KT*C]: whole tensor, one line per partition."""
    kt = a.shape[0] // P
    c = a.shape[1]
    return np.ascontiguousarray(
        a.reshape(kt, P, c).transpose(1, 0, 2).reshape(P, kt * c))


def _build(repeat=None, cc_copy=False):
    """Emit the SPMD program. repeat=None -> single pass (the real kernel).
    repeat=R wraps the phase body in a hardware loop for timing.
    cc_copy=True replaces the AllReduce with a local DRAM copy (timing-only:
    collectives inside a hardware loop desync the mesh)."""
    import concourse.bass as bass
    import concourse.mybir as mybir
    import concourse.tile as tile
    from concourse import bacc

    f32 = mybir.dt.float32
    f32r = mybir.dt.float32r
    bf16 = mybir.dt.bfloat16

    nc = bacc.Bacc("TRN2", target_bir_lowering=False)

    xT = nc.dram_tensor("xT", [NG_C, P, G_C * Bc], bf16, kind="ExternalInput")
    xfm = nc.dram_tensor("xfm", [NG_A, P, G_A * Bc], bf16, kind="ExternalInput")
    wT = nc.dram_tensor("wT", [NG_C, P, G_C * Oc], bf16, kind="ExternalInput")
    embj = nc.dram_tensor("embj", [P, KT_A * D], bf16, kind="ExternalInput")
    w1T = nc.dram_tensor("w1T", [P, 2 * D], f32, kind="ExternalInput")
    w2T = nc.dram_tensor("w2T", [P, 2 * Oc], f32, kind="ExternalInput")
    b1 = nc.dram_tensor("b1", [D], f32, kind="ExternalInput")
    b2 = nc.dram_tensor("b2", [Oc], f32, kind="ExternalInput")
    lb = nc.dram_tensor("lb", [Oc], f32, kind="ExternalInput")
    out_lin = nc.dram_tensor("out_lin", [P, 4 * Oc], bf16, kind="ExternalOutput")
    out_int = nc.dram_tensor("out_int", [P, 4 * Oc], bf16, kind="ExternalOutput")
    out_sum = nc.dram_tensor("out_sum", [P, 4 * Oc], bf16, kind="ExternalOutput")

    xT_t = xT.rearrange("kg p c -> p kg c")
    xfm_t = xfm.rearrange("kg p c -> p kg c")
    wT_t = wT.rearrange("kg p c -> p kg c")

    with tile.TileContext(nc) as tc:
        with (
            tc.tile_pool(name="const", bufs=1) as const,
            tc.tile_pool(name="dram", bufs=1, space="DRAM") as dram,
        ):
            ones_f = const.tile([1, P], f32, name="ones_f")
            nc.vector.memset(ones_f[:], 1.0)
            ones = const.tile([1, P], f32r, name="ones")
            nc.vector.tensor_copy(ones[:], ones_f[:])
            b1t = const.tile([P, 2], f32, name="b1t")
            nc.sync.dma_start(out=b1t[:], in_=b1.rearrange("(t p) -> p t", p=P))
            b2row = const.tile([1, Oc], f32r, name="b2row")
            nc.gpsimd.dma_start(out=b2row[:], in_=b2.rearrange("(a o) -> a o", a=1))
            lbrow = const.tile([1, Oc], f32r, name="lbrow")
            nc.gpsimd.dma_start(out=lbrow[:], in_=lb.rearrange("(a o) -> a o", a=1))
            # persistent SBUF intermediates (written mid-C, read in B)
            keep = const
            lin = keep.tile([P, 4 * Oc], f32, name="lin")
            lin_bf = keep.tile([P, 4 * Oc], bf16, name="lin_bf")
            intb = keep.tile([P, 4 * Oc], bf16, name="intb")
            sumb = keep.tile([P, 4 * Oc], bf16, name="sumb")
            w1s = keep.tile([P, 2 * D], f32r, name="w1s")
            w2s = keep.tile([P, 2 * Oc], f32r, name="w2s")
            red = keep.tile([P, 4 * Bc], f32, name="red")
            t1 = keep.tile([P, 2 * Bc], f32, name="t1")
            iv = keep.tile([P, 2 * Bc], f32r, name="iv")
            hsb = keep.tile([P, 2 * Bc], f32r, name="hsb")
            ccin = dram.tile([P, 4 * Bc], f32, name="ccin")
            ccout = dram.tile([P, 4 * Bc], f32, name="ccout")
            # one-time MLP weight prefetch (gpsimd: f32->f32r casts)
            nc.gpsimd.dma_start(out=w1s[:], in_=w1T[:, :])
            nc.gpsimd.dma_start(out=w2s[:], in_=w2T[:, :])

            def phase_A():
                """FM partial sums over this core's F-slice -> ccin."""
                with (
                    tc.tile_pool(name="emb_pool", bufs=1) as emb_pool,
                    tc.tile_pool(name="xf_pool", bufs=3) as xf_pool,
                    tc.tile_pool(name="x2_pool", bufs=2) as x2_pool,
                    tc.tile_pool(name="psA", bufs=1, space="PSUM") as psA,
                    tc.tile_pool(name="evA", bufs=1) as evA,
                ):
                    embt = emb_pool.tile([P, KT_A * D], bf16, name="embt")
                    nc.gpsimd.dma_start(out=embt[:], in_=embj[:, :])
                    emb2t = emb_pool.tile([P, KT_A * D], bf16, name="emb2t")
                    nc.vector.tensor_mul(emb2t[:], embt[:], embt[:])
                    se = [psA.tile([P, Bc], f32, tag=f"se{mt}", name=f"se{mt}") for mt in range(2)]
                    sq = [psA.tile([P, Bc], f32, tag=f"sq{mt}", name=f"sq{mt}") for mt in range(2)]
                    for kg in range(NG_A):
                        xf = xf_pool.tile([P, G_A * Bc], bf16, tag="xf", name="xf")
                        nc.gpsimd.dma_start(out=xf[:], in_=xfm_t[:, kg])
                        x2 = x2_pool.tile([P, G_A * Bc], bf16, tag="x2", name="x2")
                        nc.vector.tensor_mul(x2[:], xf[:], xf[:])
                        for g in range(G_A):
                            kt = kg * G_A + g
                            st, sp = kt == 0, kt == KT_A - 1
                            rs = slice(g * Bc, (g + 1) * Bc)
                            for mt in range(2):
                                ls = slice(kt * D + mt * P, kt * D + (mt + 1) * P)
                                nc.tensor.matmul(se[mt][:], embt[:, ls], xf[:, rs],
                                                 start=st, stop=sp)
                                nc.tensor.matmul(sq[mt][:], emb2t[:, ls], x2[:, rs],
                                                 start=st, stop=sp)
                    ev = evA.tile([P, 4 * Bc], f32, name="ev")
                    for mt in range(2):
                        nc.vector.tensor_copy(ev[:, mt * Bc:(mt + 1) * Bc], se[mt][:])
                        nc.vector.tensor_copy(ev[:, (2 + mt) * Bc:(3 + mt) * Bc], sq[mt][:])
                    nc.scalar.dma_start(out=ccin[:], in_=ev[:])

            def emit_reduce():
                """AllReduce (or its timing stand-in) + reduced-sum fetch and
                the iv computation — emitted a few groups into phase C so the
                gpsimd x-stream is never parked behind the collective."""
                if cc_copy:
                    nc.gpsimd.dma_start(out=ccout[:], in_=ccin[:])
                else:
                    import concourse.mybir as mybir
                    nc.gpsimd.collective_compute(
                        "AllReduce", mybir.AluOpType.add, replica_groups=GROUPS,
                        ins=[ccin.opt()], outs=[ccout.opt()],
                    )
                nc.scalar.dma_start(out=red[:], in_=ccout[:])

            def emit_iv():
                """iv = 0.5*(sum_emb^2 - sum_sq), on the idle vector engine."""
                se_r, sq_r = red[:, 0:2 * Bc], red[:, 2 * Bc:4 * Bc]
                nc.vector.tensor_mul(t1[:], se_r, se_r)
                nc.vector.tensor_sub(t1[:], t1[:], sq_r)
                nc.vector.tensor_scalar_mul(iv[:], t1[:], 0.5)

            def phase_C(xt_pool, wt_pool):
                """linear = x @ lin_w.T + lin_b -> lin (f32) + out_lin."""
                with tc.tile_pool(name="psC", bufs=1, space="PSUM") as psC:
                    ps = [[psC.tile([P, 512], f32, tag=f"ps{m}{n}", name=f"ps{m}{n}")
                           for n in range(2)] for m in range(4)]
                    for m in range(4):
                        for n in range(2):
                            nc.tensor.matmul(ps[m][n][:], ones[:],
                                             lbrow[:, n * 512:(n + 1) * 512],
                                             start=True, stop=False)
                    for kg in range(NG_C):
                        xt = xt_pool.tile([P, G_C * Bc], bf16, tag="xt", name="xt")
                        wt = wt_pool.tile([P, G_C * Oc], bf16, tag="wt", name="wt")
                        nc.gpsimd.dma_start(out=xt[:], in_=xT_t[:, kg])
                        nc.sync.dma_start(out=wt[:], in_=wT_t[:, kg])
                        if kg == 3:
                            emit_reduce()
                        if kg == 4:
                            emit_iv()
                        for g in range(G_C):
                            k = kg * G_C + g
                            for m in range(4):
                                lhsT = xt[:, g * Bc + m * P:g * Bc + (m + 1) * P]
                                for n in range(2):
                                    nc.tensor.matmul(ps[m][n][:], lhsT,
                                                     wt[:, g * Oc + n * 512:g * Oc + (n + 1) * 512],
                                                     start=False, stop=(k == KT_C - 1))
                    for m in range(4):
                        for n in range(2):
                            sl = slice(m * Oc + n * 512, m * Oc + (n + 1) * 512)
                            nc.vector.tensor_copy(lin[:, sl], ps[m][n][:])
                            # Pool can't read PSUM; downcast from the SBUF copy
                            nc.gpsimd.tensor_copy(lin_bf[:, sl], lin[:, sl])
                        nc.sync.dma_start(out=out_lin[:, m * Oc:(m + 1) * Oc],
                                          in_=lin_bf[:, m * Oc:(m + 1) * Oc])

            def phase_B():
                """h = relu(iv@w1.T+b1); inter = h@w2.T + b2 -> out_int/out_sum."""
                import concourse.mybir as mybir
                with tc.tile_pool(name="psB", bufs=2, space="PSUM") as psB:
                    for mt in range(2):
                        hp = psB.tile([P, Bc], f32, tag="hp", name="hp")
                        for kt in range(2):
                            nc.tensor.matmul(hp[:], w1s[:, kt * D + mt * P:kt * D + (mt + 1) * P],
                                             iv[:, kt * Bc:(kt + 1) * Bc],
                                             start=(kt == 0), stop=(kt == 1))
                        nc.scalar.activation(hsb[:, mt * Bc:(mt + 1) * Bc], hp[:],
                                             mybir.ActivationFunctionType.Relu,
                                             bias=b1t[:, mt:mt + 1])
                    for mb in range(4):
                        for no in range(2):
                            pi = psB.tile([P, 512], f32, tag="pi", name="pi", bufs=4)
                            nc.tensor.matmul(pi[:], ones[:],
                                             b2row[:, no * 512:(no + 1) * 512],
                                             start=True, stop=False)
                            for kt in range(2):
                                nc.tensor.matmul(pi[:], hsb[:, kt * Bc + mb * P:kt * Bc + mb * P + P],
                                                 w2s[:, kt * Oc + no * 512:kt * Oc + (no + 1) * 512],
                                                 start=False, stop=(kt == 1))
                            sl = slice(mb * Oc + no * 512, mb * Oc + (no + 1) * 512)
                            nc.scalar.copy(intb[:, sl], pi[:])
                            nc.vector.tensor_add(sumb[:, sl], pi[:], lin[:, sl])
                        osl = slice(mb * Oc, (mb + 1) * Oc)
                        # sync + scalar queues: keep gpsimd free so the next
                        # iteration's embt/xf prefetch is never parked here
                        nc.sync.dma_start(out=out_int[:, osl], in_=intb[:, osl])
                        nc.scalar.dma_start(out=out_sum[:, osl], in_=sumb[:, osl])

            def body():
                # the C stream pools live OUTSIDE phase A's pools so their
                # SBUF ranges don't alias A's tiles — otherwise the wt/xt
                # prefetch DMAs stall until A's last embt read
                with (
                    tc.tile_pool(name="xt_pool", bufs=4) as xt_pool,
                    tc.tile_pool(name="wt_pool", bufs=4) as wt_pool,
                ):
                    phase_A()
                    phase_C(xt_pool, wt_pool)  # emits AllReduce + iv mid-loop
                    phase_B()

            if repeat is None:
                body()
            else:
                import concourse.mybir as _mb
                with tc.For_i(0, repeat, 1, hint_engines=(_mb.EngineType.PE,)) as _i:
                    body()
    nc.compile()
    return nc


def _prep_inputs(sae_features, emb, lin_w, lin_b, w1, b1, w2, b2):
    """Host-side shard + transpose + bf16 cast + DMA-friendly permutes."""
    x = np.asarray(sae_features, dtype=np.float32)
    emb = np.asarray(emb, dtype=np.float32).astype(BF16)
    lin_w = np.asarray(lin_w, dtype=np.float32)
    w1T = _part_rows(np.ascontiguousarray(np.asarray(w1, np.float32).T))
    w2 = np.asarray(w2, dtype=np.float32)
    b1 = np.asarray(b1, np.float32)
    b2 = np.asarray(b2, np.float32)
    lin_b = np.asarray(lin_b, np.float32)

    xT_half = [np.ascontiguousarray(x[g * Bc:(g + 1) * Bc, :].T).astype(BF16)
               for g in range(2)]
    xTg = [_group_rows(h, G_C) for h in xT_half]
    xfmg = [[_group_rows(h[j * Fj:(j + 1) * Fj], G_A) for j in range(4)]
            for h in xT_half]
    wTg = [_group_rows(np.ascontiguousarray(lin_w[j * Oc:(j + 1) * Oc, :].T).astype(BF16), G_C)
           for j in range(4)]
    embg = [_part_rows(np.ascontiguousarray(emb[j * Fj:(j + 1) * Fj])) for j in range(4)]
    w2Tg = [_part_rows(np.ascontiguousarray(w2[j * Oc:(j + 1) * Oc, :].T)) for j in range(4)]
    in_maps = []
    for c in range(NCORES):
        g, j = c // 4, c % 4
        in_maps.append({
            "xT": xTg[g],
            "xfm": xfmg[g][j],
            "wT": wTg[j],
            "embj": embg[j],
            "w1T": w1T,
            "w2T": w2Tg[j],
            "b1": b1,
            "b2": np.ascontiguousarray(b2[j * Oc:(j + 1) * Oc]),
            "lb": np.ascontiguousarray(lin_b[j * Oc:(j + 1) * Oc]),
        })
    return in_maps


def _gather(results):
    """Assemble full fp32 outputs from per-core [P, 4*Oc] bf16 blocks."""
    outs = {}
    for key in ("out_sum", "out_lin", "out_int"):
        full = np.empty((B, O), dtype=np.float32)
        for c in range(NCORES):
            g, j = c // 4, c % 4
            blk = np.asarray(results[c][key], dtype=np.float32)
            blk = blk.reshape(P, 4, Oc).transpose(1, 0, 2).reshape(Bc, Oc)
            full[g * Bc:(g + 1) * Bc, j * Oc:(j + 1) * Oc] = blk
        outs[key] = full
    return outs["out_sum"], outs["out_lin"], outs["out_int"]


def kernel(sae_features, emb, lin_w, lin_b, w1, b1, w2, b2):
    from concourse.bass_utils import run_bass_kernel_spmd

    if "nc" not in _cache:
        _cache["nc"] = _build()
    nc = _cache["nc"]
    in_maps = _prep_inputs(sae_features, emb, lin_w, lin_b, w1, b1, w2, b2)
    try:
        res = run_bass_kernel_spmd(nc, in_maps, list(range(NCORES)))
    except Exception:
        # transient device desync/unrecoverable states heal on retry
        import time as _time
        _time.sleep(5)
        res = run_bass_kernel_spmd(nc, in_maps, list(range(NCORES)))
    return _gather(res.results)
